# revision 12
# baseline (speedup 1.0000x reference)
"""Sparse dual-masked attention for Trainium2, 8 NeuronCores.

Problem: B=2, N=2048, DIM=512, H=8, DH=64.
  qkv = x @ W_qkv; per-head attention with dual mask
  (np_i*np_j==0 | bert_j==1 -> -1000), softmax, out proj + bias.

Key structure exploited (sparse_attention):
  - A row i with np_i==0 is fully masked -> softmax is uniform -> output row
    is the constant mean(V) @ W_out + b_out (computed on host; tiny).
  - For rows with np_i==1, only columns with np_j==1 & bert_j==0 survive
    (exp(-1000-max) == 0 exactly in the reference). So we gather those
    ~R=1030 rows and ~M=535 columns on the host and run a dense attention
    over the gathered set on device: ~8x less work than dense.

Sharding: core = (batch b, head-pair g): 2 batches x 4 head groups.
  W_qkv is split column-wise per head pair, W_out row-wise; each core
  produces a partial [R,512] output; host sums the 4 partials per batch.

All matmul operands are bf16 (1 cyc/col on the PE at any moving size and
fast-weight-load eligible, vs 4 cyc/col + slow LDWEIGHTS for fp32);
accumulation stays fp32 in PSUM.  End-to-end error ~4e-3 against the
2e-2 correctness gate.  The two heads' S matmuls have 64-deep
contractions at PE row groups 0/64 (row tiling).  A burst of dummy
matmuls on a memset scratch tile warms the PE's HAM clock gate (cold PE
runs at 1.2 GHz, warm 2.4) with no DMA dependency, covering the input
DMA window.  All static weights ship as one host-packed blob (single
DMA trigger; triggers cost ~650ns each on an engine queue and
completion semaphores are a shared pool of 8).

Device dataflow per core (R=R_PAD rows, M=M_PAD kv cols, 2 heads):
  1. K^T = Wk^T x^T  [128, M] (PSUM accum over 4 c-chunks), Q^T likewise
     per r-chunk; V = x^T^T Wv [m-tile, 128] per m-tile.  V_aug per
     m-tile [128, 256]: per head [kv-indicator replicated x64 | V(64)],
     rows scaled by the indicator to null tail rows in [M, M_PAD).
  2. S^T[h] = K_h Q_h^T  [m-tile 128, r-chunk] pairs into one 2-bank
     PSUM tile; exp of both heads in a single Act instruction -> PT.
  3. O^T[h] = V_aug_h^T P^T [128, r-chunk] accumulated over m-tiles;
     partitions 0:64 carry the softmax denominator (already replicated,
     courtesy of the kv block in V_aug) -> reciprocal (DVE) -> fused
     normalize (tensor_mul) into OnT.
  4. y = OnT^T @ W_out_rows [R, 512] -> bf16 -> DMA out (host upcasts,
     sums partials, adds bias), interleaved with phase 3 so output DMAs
     overlap compute.
"""

import numpy as np

_CORES = 8
_DIM = 512
_DH = 64
_H = 8
_INNER = _H * _DH
_WARMUP_MMS = 48


def _ceil_to(x, m):
    return ((x + m - 1) // m) * m


def _chunks(total, step):
    out = []
    o = 0
    while o < total:
        out.append((o, min(step, total - o)))
        o += step
    return out


def _chunks_ge(total, step=512, minc=256):
    """Chunks of <= step, each >= minc (rebalancing the tail)."""
    out = _chunks(total, step)
    if len(out) >= 2 and out[-1][1] < minc:
        o_prev, w_prev = out[-2]
        o_last, w_last = out[-1]
        move = minc - w_last
        out[-2] = (o_prev, w_prev - move)
        out[-1] = (o_last - move, w_last + move)
    return out


def build_bass(R_PAD, M_PAD):
    """Build the SPMD bass program for padded sizes R_PAD (queries) and
    M_PAD (kv columns). Returns the compiled Bacc object."""
    import concourse.bacc as bacc
    import concourse.mybir as mybir
    import concourse.tile as tile

    f32 = mybir.dt.float32
    bf16 = mybir.dt.bfloat16
    EXP = mybir.ActivationFunctionType.Exp

    assert R_PAD % 16 == 0 and M_PAD % 128 == 0 and R_PAD >= M_PAD
    NMT = M_PAD // 128          # kv m-tiles
    NRT = (R_PAD + 127) // 128  # query r-tiles for the final projection
    RC = _chunks_ge(R_PAD)      # <=512 keeps each S tile in one PSUM bank
    MC = _chunks_ge(M_PAD)
    assert len(RC) <= 3

    # weight blob layout (bf16 columns)
    WQ, WK, WV, WO, KVR = 0, 512, 1024, 1536, 2048
    WB_COLS = KVR + NMT * 64

    nc = bacc.Bacc("TRN2", target_bir_lowering=False, debug=False,
                   num_devices=_CORES)

    xT_d = nc.dram_tensor("xT", [128, 4, R_PAD], bf16, kind="ExternalInput")
    wb_d = nc.dram_tensor("wb", [128, WB_COLS], bf16, kind="ExternalInput")
    kvc_d = nc.dram_tensor("kvc", [128, NMT], f32, kind="ExternalInput")
    y_d = nc.dram_tensor("y", [R_PAD, 512], bf16, kind="ExternalOutput")

    with tile.TileContext(nc) as tc:
        with (
            tc.tile_pool(name="sb", bufs=1) as sb,
            tc.tile_pool(name="pp", bufs=2, space="PSUM") as pp,
        ):
            # ---- input DMAs: one packed weight blob + kvc on the sync
            # queue; xT split c-chunk x col-range across all three DMA-
            # capable queues so the K/V/S columns land first -------------
            xT = sb.tile([128, 4, R_PAD], bf16, tag="xT")
            nc.scalar.dma_start(out=xT[:, 0:2, :], in_=xT_d.ap()[:, 0:2, :])
            nc.gpsimd.dma_start(out=xT[:, 2:4, :], in_=xT_d.ap()[:, 2:4, :])
            wb = sb.tile([128, WB_COLS], bf16, tag="wb")
            nc.sync.dma_start(out=wb, in_=wb_d.ap())
            kvc = sb.tile([128, NMT], f32, tag="kvc")
            nc.sync.dma_start(out=kvc, in_=kvc_d.ap())

            def wqc(c):
                return wb[:, WQ + c * 128:WQ + c * 128 + 128]

            def wkc(c):
                return wb[:, WK + c * 128:WK + c * 128 + 128]

            def wvc(c):
                return wb[:, WV + c * 128:WV + c * 128 + 128]

            wo = wb[:, WO:WO + 512]

            def kvr(mt):
                return wb[:, KVR + mt * 64:KVR + mt * 64 + 64]

            # ---- PE warmup: dummy matmuls on a memset scratch tile (no
            # DMA dependency) warm the HAM clock gate during the input DMA
            # window so real matmuls run at 2.4 GHz -----------------------
            junk = sb.tile([128, 128], bf16, tag="junk")
            nc.vector.memset(junk, 0.25)
            warm = pp.tile([128, 512], f32, tag="o", name="warm")
            for _ in range(_WARMUP_MMS):
                nc.tensor.matmul(warm[:, 0:128], junk, junk,
                                 start=True, stop=True)

            # ---- phase 1: projections (K first: S needs all of K but only
            # the first r-chunk of Q) -------------------------------------
            KT = sb.tile([128, M_PAD], bf16, tag="KT")
            for i, (o, w) in enumerate(MC):
                kps = pp.tile([128, 512], f32, tag="big", name=f"kps{i}")
                for c in range(4):
                    nc.tensor.matmul(kps[:, :w], wkc(c), xT[:, c, o:o + w],
                                     start=(c == 0), stop=(c == 3))
                nc.scalar.copy(KT[:, o:o + w], kps[:, :w])

            QT = sb.tile([128, R_PAD], bf16, tag="QT")

            def emit_Q(i):
                o, w = RC[i]
                qps = pp.tile([128, 512], f32, tag="big", name=f"qps{i}")
                for c in range(4):
                    nc.tensor.matmul(qps[:, :w], wqc(c), xT[:, c, o:o + w],
                                     start=(c == 0), stop=(c == 3))
                nc.scalar.copy(QT[:, o:o + w], qps[:, :w])

            emit_Q(0)

            # V + V_aug: [kv-indicator replicated x64 | V(64)] per head so
            # the O matmul emits the softmax denominator replicated across
            # partitions 0:64 (normalization then needs no replicate step)
            V = []
            for g, mts in enumerate([list(range(min(4, NMT))),
                                     list(range(4, NMT))]):
                if not mts:
                    continue
                vg = pp.tile([128, 512], f32, tag="big", name=f"vg{g}")
                for j, mt in enumerate(mts):
                    msl = slice(mt * 128, (mt + 1) * 128)
                    for c in range(4):
                        nc.tensor.matmul(vg[:, j * 128:(j + 1) * 128],
                                         xT[:, c, msl], wvc(c),
                                         start=(c == 0), stop=(c == 3))
                for j, mt in enumerate(mts):
                    vt = sb.tile([128, 256], bf16, tag=f"v{mt}",
                                 name=f"v{mt}")
                    nc.gpsimd.tensor_copy(vt[:, 0:64], kvr(mt))
                    nc.gpsimd.tensor_copy(vt[:, 128:192], kvr(mt))
                    nc.vector.tensor_scalar_mul(
                        vt[:, 64:128], in0=vg[:, j * 128:j * 128 + 64],
                        scalar1=kvc[:, mt:mt + 1])
                    nc.vector.tensor_scalar_mul(
                        vt[:, 192:256], in0=vg[:, j * 128 + 64:j * 128 + 128],
                        scalar1=kvc[:, mt:mt + 1])
                    V.append(vt)

            # ---- phase 2+3, software-pipelined over r-chunks -------------
            PT = [sb.tile([128, 2, R_PAD], bf16, tag="pt", bufs=NMT,
                          name=f"pt{mt}") for mt in range(NMT)]
            OnT = sb.tile([128, R_PAD], bf16, tag="OnT")
            ystate = [0]

            def emit_S(i):
                o, w = RC[i]
                for mt in range(NMT):
                    msl = slice(mt * 128, (mt + 1) * 128)
                    pair = pp.tile([128, 2, 512], f32, tag="pair")
                    for h in range(2):
                        hs = slice(h * 64, (h + 1) * 64)
                        nc.tensor.matmul(pair[:, h, :w], KT[hs, msl],
                                         QT[hs, o:o + w],
                                         start=True, stop=True)
                    nc.scalar.activation(out=PT[mt][:, :, o:o + w],
                                         in_=pair[:, :, :w], func=EXP)

            def emit_O(i):
                o, w = RC[i]
                for h in range(2):
                    vs = slice(h * 128, (h + 1) * 128)
                    ops = pp.tile([128, 512], f32, tag="o")
                    for mt in range(NMT):
                        nc.tensor.matmul(ops[:, :w], V[mt][:, vs],
                                         PT[mt][:, h, o:o + w],
                                         start=(mt == 0), stop=(mt == NMT - 1))
                    rcp = sb.tile([64, 512], f32, tag="rcp", bufs=4)
                    nc.vector.reciprocal_approx_fast(rcp[:, :w], ops[0:64, :w])
                    nc.vector.tensor_mul(OnT[h * 64:(h + 1) * 64, o:o + w],
                                         ops[64:128, :w], rcp[:, :w])
                # out projection for the r-tiles fully covered so far
                done = o + w
                while ystate[0] < NRT and min(ystate[0] * 128 + 128,
                                              R_PAD) <= done:
                    rt = ystate[0]
                    tw = min(128, R_PAD - rt * 128)
                    ps = pp.tile([128, 512], f32, tag="big")
                    rsl = slice(rt * 128, rt * 128 + tw)
                    nc.tensor.matmul(ps[:tw, :], OnT[:, rsl], wo,
                                     start=True, stop=True)
                    ysb = sb.tile([128, 512], bf16, tag="y", bufs=5)
                    nc.vector.tensor_copy(ysb[:tw, :], ps[:tw, :])
                    (nc.sync if rt % 2 == 0 else nc.gpsimd).dma_start(
                        out=y_d.ap()[rsl, :], in_=ysb[:tw, :])
                    ystate[0] += 1

            emit_S(0)
            for i in range(1, len(RC)):
                emit_Q(i)
                emit_S(i)
                emit_O(i - 1)
            emit_O(len(RC) - 1)

    nc.compile()
    return nc


def _prep(x, mask_np, mask_bert, W_qkv, W_out):
    """Host-side gather/shard. Returns (in_maps, meta)."""
    import ml_dtypes

    bf16 = ml_dtypes.bfloat16
    B, N, DIM = x.shape
    assert (B, DIM) == (2, _DIM)
    x = np.ascontiguousarray(x, dtype=np.float32)
    W_qkv = np.ascontiguousarray(W_qkv, dtype=np.float32)
    W_out = np.ascontiguousarray(W_out, dtype=np.float32)

    kv_idx, tail_idx, Ms, tails = [], [], [], []
    for b in range(B):
        npb = mask_np[b].astype(bool)
        bb = mask_bert[b].astype(bool)
        kv = np.nonzero(npb & ~bb)[0]
        tl = np.nonzero(npb & bb)[0]
        kv_idx.append(kv)
        tail_idx.append(tl)
        Ms.append(len(kv))
        tails.append(len(tl))

    M_PAD = max(128, _ceil_to(max(Ms), 128))
    # rows are packed [kv | tail] with no gap: the tail rows that fall in
    # [M_b, M_PAD) act as key/value candidates but are nulled by the kv
    # indicator (V rows scaled to 0, denominator contribution 0), so no
    # zero gap is needed and R_PAD shrinks to the real row count.
    R_PAD = max(128, _ceil_to(max(Ms[b] + tails[b] for b in range(B)), 16),
                M_PAD)

    NMT = M_PAD // 128
    xT_b, kvc_b, kvr_b, row_pos = [], [], [], []
    for b in range(B):
        xa = np.zeros((512, R_PAD), dtype=bf16)
        xa[:, :Ms[b]] = x[b][kv_idx[b]].T
        xa[:, Ms[b]:Ms[b] + tails[b]] = x[b][tail_idx[b]].T
        # pack into the on-chip tile layout [p, c, r] = xa[c*128+p, r] so
        # the DMA moves 4KB+ contiguous runs per partition
        xT_b.append(np.ascontiguousarray(
            xa.reshape(4, 128, R_PAD).transpose(1, 0, 2)))
        kvones = np.zeros(M_PAD, dtype=np.float32)
        kvones[:Ms[b]] = 1.0
        km = np.ascontiguousarray(kvones.reshape(NMT, 128).T)  # [128, NMT]
        kvc_b.append(km)
        kvr_b.append(np.repeat(km[:, :, None], 64, axis=2)
                     .reshape(128, NMT * 64).astype(bf16))
        # output row p of the device result corresponds to token row_pos[p]
        pos = np.concatenate([kv_idx[b], tail_idx[b]])
        row_pos.append(pos)

    def shuffle(wmat):
        # [512, 128] -> [128, 4*128] with [p, c*128+d] = wmat[c*128+p, d]
        return wmat.reshape(4, 128, 128).transpose(1, 0, 2).reshape(128, 512)

    scale = np.float32(_DH ** -0.5)
    in_maps = []
    for c in range(_CORES):
        b, g = divmod(c, 4)
        qc = slice(128 * g, 128 * g + 128)
        kc = slice(_INNER + 128 * g, _INNER + 128 * g + 128)
        vc = slice(2 * _INNER + 128 * g, 2 * _INNER + 128 * g + 128)
        wb = np.concatenate([
            shuffle(W_qkv[:, qc] * scale),
            shuffle(W_qkv[:, kc]),
            shuffle(W_qkv[:, vc]),
            W_out[128 * g:128 * g + 128, :],
            kvr_b[b].astype(np.float32),
        ], axis=1).astype(bf16)
        in_maps.append({"xT": xT_b[b], "wb": np.ascontiguousarray(wb),
                        "kvc": kvc_b[b]})

    meta = dict(M_PAD=M_PAD, R_PAD=R_PAD, Ms=Ms, tails=tails,
                kv_idx=kv_idx, tail_idx=tail_idx, row_pos=row_pos)
    return in_maps, meta


def _assemble(results, meta, x, mask_np, W_qkv, W_out, b_out):
    B, N, _ = x.shape
    out = np.empty((B, N, _DIM), dtype=np.float32)
    Wv_full = W_qkv[:, 2 * _INNER:].astype(np.float32)
    for b in range(B):
        # constant output for fully-masked rows: uniform attention = mean(V)
        meanv = (x[b].mean(axis=0, dtype=np.float32) @ Wv_full)
        yconst = meanv @ W_out.astype(np.float32) + b_out
        out[b, :, :] = yconst[None, :]
        Mb, tb = meta["Ms"][b], meta["tails"][b]
        if Mb == 0:
            # no unmasked kv columns: every row is fully masked -> uniform
            continue
        acc = None
        for g in range(4):
            yp = np.asarray(results[4 * b + g]["y"], dtype=np.float32)
            acc = yp if acc is None else acc + yp
        out[b, meta["row_pos"][b], :] = acc[:Mb + tb] + b_out
    return out


_CACHE = {}


def _get_bass(R_PAD, M_PAD):
    key = (R_PAD, M_PAD)
    if key not in _CACHE:
        _CACHE[key] = build_bass(R_PAD, M_PAD)
    return _CACHE[key]


def run_spmd(in_maps, meta, trace=False, tmpdir=None, trace_cores=None):
    from concourse.bass_utils import run_bass_kernel_spmd

    nc = _get_bass(meta["R_PAD"], meta["M_PAD"])
    return run_bass_kernel_spmd(
        nc, in_maps, core_ids=list(range(_CORES)), trace=trace, tmpdir=tmpdir,
        trace_cores=trace_cores)


def kernel(x, mask_np, mask_bert, W_qkv, W_out, b_out):
    x = np.asarray(x)
    mask_np = np.asarray(mask_np)
    mask_bert = np.asarray(mask_bert)
    W_qkv = np.asarray(W_qkv, dtype=np.float32)
    W_out = np.asarray(W_out, dtype=np.float32)
    b_out = np.asarray(b_out, dtype=np.float32)

    in_maps, meta = _prep(x, mask_np, mask_bert, W_qkv, W_out)
    res = run_spmd(in_maps, meta)
    return _assemble(res.results, meta, x, mask_np, W_qkv, W_out, b_out)


# revision 13
# speedup vs baseline: 1.0114x; 1.0114x over previous
"""Sparse dual-masked attention for Trainium2, 8 NeuronCores.

Problem: B=2, N=2048, DIM=512, H=8, DH=64.
  qkv = x @ W_qkv; per-head attention with dual mask
  (np_i*np_j==0 | bert_j==1 -> -1000), softmax, out proj + bias.

Key structure exploited (sparse_attention):
  - A row i with np_i==0 is fully masked -> softmax is uniform -> output row
    is the constant mean(V) @ W_out + b_out (computed on host; tiny).
  - For rows with np_i==1, only columns with np_j==1 & bert_j==0 survive
    (exp(-1000-max) == 0 exactly in the reference). So we gather those
    ~R=1030 rows and ~M=535 columns on the host and run a dense attention
    over the gathered set on device: ~8x less work than dense.

Sharding: core = (batch b, head-pair g): 2 batches x 4 head groups.
  W_qkv is split column-wise per head pair, W_out row-wise; each core
  produces a partial [R,512] output; host sums the 4 partials per batch.

All matmul operands are bf16 (1 cyc/col on the PE at any moving size and
fast-weight-load eligible, vs 4 cyc/col + slow LDWEIGHTS for fp32);
accumulation stays fp32 in PSUM.  End-to-end error ~4e-3 against the
2e-2 correctness gate.  The two heads' S matmuls have 64-deep
contractions at PE row groups 0/64 (row tiling).  A burst of dummy
matmuls on a memset scratch tile warms the PE's HAM clock gate (cold PE
runs at 1.2 GHz, warm 2.4) with no DMA dependency, covering the input
DMA window.  All static weights ship as one host-packed blob (single
DMA trigger; triggers cost ~650ns each on an engine queue and
completion semaphores are a shared pool of 8).

Device dataflow per core (R=R_PAD rows, M=M_PAD kv cols, 2 heads):
  1. K^T = Wk^T x^T  [128, M] (PSUM accum over 4 c-chunks), Q^T likewise
     per r-chunk; V = x^T^T Wv [m-tile, 128] per m-tile.  V_aug per
     m-tile [128, 256]: per head [kv-indicator replicated x64 | V(64)],
     rows scaled by the indicator to null tail rows in [M, M_PAD).
  2. S^T[h] = K_h Q_h^T  [m-tile 128, r-chunk] pairs into one 2-bank
     PSUM tile; exp of both heads in a single Act instruction -> PT.
  3. O^T[h] = V_aug_h^T P^T [128, r-chunk] accumulated over m-tiles;
     partitions 0:64 carry the softmax denominator (already replicated,
     courtesy of the kv block in V_aug) -> reciprocal (DVE) -> fused
     normalize (tensor_mul) into OnT.
  4. y = OnT^T @ W_out_rows [R, 512] -> bf16 -> DMA out (host upcasts,
     sums partials, adds bias), interleaved with phase 3 so output DMAs
     overlap compute.
"""

import numpy as np

_CORES = 8
_DIM = 512
_DH = 64
_H = 8
_INNER = _H * _DH
_WARMUP_MMS = 40


def _ceil_to(x, m):
    return ((x + m - 1) // m) * m


def _chunks(total, step):
    out = []
    o = 0
    while o < total:
        out.append((o, min(step, total - o)))
        o += step
    return out


def _chunks_ge(total, step=512, minc=256):
    """Chunks of <= step, each >= minc (rebalancing the tail)."""
    out = _chunks(total, step)
    if len(out) >= 2 and out[-1][1] < minc:
        o_prev, w_prev = out[-2]
        o_last, w_last = out[-1]
        move = minc - w_last
        out[-2] = (o_prev, w_prev - move)
        out[-1] = (o_last - move, w_last + move)
    return out


def build_bass(R_PAD, M_PAD):
    """Build the SPMD bass program for padded sizes R_PAD (queries) and
    M_PAD (kv columns). Returns the compiled Bacc object."""
    import concourse.bacc as bacc
    import concourse.mybir as mybir
    import concourse.tile as tile

    f32 = mybir.dt.float32
    bf16 = mybir.dt.bfloat16
    EXP = mybir.ActivationFunctionType.Exp

    assert R_PAD % 16 == 0 and M_PAD % 128 == 0 and R_PAD >= M_PAD
    NMT = M_PAD // 128          # kv m-tiles
    NRT = (R_PAD + 127) // 128  # query r-tiles for the final projection
    RC = _chunks_ge(R_PAD)      # <=512 keeps each S tile in one PSUM bank
    MC = _chunks_ge(M_PAD)
    assert len(RC) <= 3

    # weight blob layouts (bf16 columns): wb1 = [wq|wk] gates the S phase
    # and ships first; wb2 = [wv|wo|kvr] is needed later (V_aug, y proj)
    WQ, WK = 0, 512
    WV, WO, KVR = 0, 512, 1024
    WB2_COLS = KVR + NMT * 64

    nc = bacc.Bacc("TRN2", target_bir_lowering=False, debug=False,
                   num_devices=_CORES)

    xT_d = nc.dram_tensor("xT", [128, 4, R_PAD], bf16, kind="ExternalInput")
    wb1_d = nc.dram_tensor("wb1", [128, 1024], bf16, kind="ExternalInput")
    wb2_d = nc.dram_tensor("wb2", [128, WB2_COLS], bf16, kind="ExternalInput")
    kvc_d = nc.dram_tensor("kvc", [128, NMT], f32, kind="ExternalInput")
    y_d = nc.dram_tensor("y", [R_PAD, 512], bf16, kind="ExternalOutput")

    with tile.TileContext(nc) as tc:
        with (
            tc.tile_pool(name="sb", bufs=1) as sb,
            tc.tile_pool(name="pp", bufs=2, space="PSUM") as pp,
        ):
            # ---- input DMAs: one packed weight blob + kvc on the sync
            # queue; xT split c-chunk x col-range across all three DMA-
            # capable queues so the K/V/S columns land first -------------
            xT = sb.tile([128, 4, R_PAD], bf16, tag="xT")
            nc.scalar.dma_start(out=xT[:, 0:2, :], in_=xT_d.ap()[:, 0:2, :])
            nc.gpsimd.dma_start(out=xT[:, 2:4, :], in_=xT_d.ap()[:, 2:4, :])
            wb1 = sb.tile([128, 1024], bf16, tag="wb1")
            nc.sync.dma_start(out=wb1, in_=wb1_d.ap())
            wb2 = sb.tile([128, WB2_COLS], bf16, tag="wb2")
            nc.sync.dma_start(out=wb2, in_=wb2_d.ap())
            kvc = sb.tile([128, NMT], f32, tag="kvc")
            nc.sync.dma_start(out=kvc, in_=kvc_d.ap())

            def wqc(c):
                return wb1[:, WQ + c * 128:WQ + c * 128 + 128]

            def wkc(c):
                return wb1[:, WK + c * 128:WK + c * 128 + 128]

            def wvc(c):
                return wb2[:, WV + c * 128:WV + c * 128 + 128]

            wo = wb2[:, WO:WO + 512]

            def kvr(mt):
                return wb2[:, KVR + mt * 64:KVR + mt * 64 + 64]

            # ---- PE warmup: dummy matmuls on a memset scratch tile (no
            # DMA dependency) warm the HAM clock gate during the input DMA
            # window so real matmuls run at 2.4 GHz -----------------------
            junk = sb.tile([128, 128], bf16, tag="junk")
            nc.vector.memset(junk, 0.25)
            warm = pp.tile([128, 512], f32, tag="o", name="warm")
            for _ in range(_WARMUP_MMS):
                nc.tensor.matmul(warm[:, 0:128], junk, junk,
                                 start=True, stop=True)

            # ---- phase 1: projections (K first: S needs all of K but only
            # the first r-chunk of Q) -------------------------------------
            KT = sb.tile([128, M_PAD], bf16, tag="KT")
            for i, (o, w) in enumerate(MC):
                kps = pp.tile([128, 512], f32, tag="big", name=f"kps{i}")
                for c in range(4):
                    nc.tensor.matmul(kps[:, :w], wkc(c), xT[:, c, o:o + w],
                                     start=(c == 0), stop=(c == 3))
                nc.scalar.copy(KT[:, o:o + w], kps[:, :w])

            QT = sb.tile([128, R_PAD], bf16, tag="QT")

            def emit_Q(i):
                o, w = RC[i]
                qps = pp.tile([128, 512], f32, tag="big", name=f"qps{i}")
                for c in range(4):
                    nc.tensor.matmul(qps[:, :w], wqc(c), xT[:, c, o:o + w],
                                     start=(c == 0), stop=(c == 3))
                nc.scalar.copy(QT[:, o:o + w], qps[:, :w])

            emit_Q(0)

            # V + V_aug: [kv-indicator replicated x64 | V(64)] per head so
            # the O matmul emits the softmax denominator replicated across
            # partitions 0:64 (normalization then needs no replicate step)
            V = []
            for g, mts in enumerate([list(range(min(4, NMT))),
                                     list(range(4, NMT))]):
                if not mts:
                    continue
                vg = pp.tile([128, 512], f32, tag="big", name=f"vg{g}")
                for j, mt in enumerate(mts):
                    msl = slice(mt * 128, (mt + 1) * 128)
                    for c in range(4):
                        nc.tensor.matmul(vg[:, j * 128:(j + 1) * 128],
                                         xT[:, c, msl], wvc(c),
                                         start=(c == 0), stop=(c == 3))
                for j, mt in enumerate(mts):
                    vt = sb.tile([128, 256], bf16, tag=f"v{mt}",
                                 name=f"v{mt}")
                    nc.gpsimd.tensor_copy(vt[:, 0:64], kvr(mt))
                    nc.gpsimd.tensor_copy(vt[:, 128:192], kvr(mt))
                    nc.vector.tensor_scalar_mul(
                        vt[:, 64:128], in0=vg[:, j * 128:j * 128 + 64],
                        scalar1=kvc[:, mt:mt + 1])
                    nc.vector.tensor_scalar_mul(
                        vt[:, 192:256], in0=vg[:, j * 128 + 64:j * 128 + 128],
                        scalar1=kvc[:, mt:mt + 1])
                    V.append(vt)

            # ---- phase 2+3, software-pipelined over r-chunks -------------
            PT = [sb.tile([128, 2, R_PAD], bf16, tag="pt", bufs=NMT,
                          name=f"pt{mt}") for mt in range(NMT)]
            OnT = sb.tile([128, R_PAD], bf16, tag="OnT")
            ystate = [0]

            def emit_S(i):
                o, w = RC[i]
                for mt in range(NMT):
                    msl = slice(mt * 128, (mt + 1) * 128)
                    pair = pp.tile([128, 2, 512], f32, tag="pair")
                    for h in range(2):
                        hs = slice(h * 64, (h + 1) * 64)
                        nc.tensor.matmul(pair[:, h, :w], KT[hs, msl],
                                         QT[hs, o:o + w],
                                         start=True, stop=True)
                    nc.scalar.activation(out=PT[mt][:, :, o:o + w],
                                         in_=pair[:, :, :w], func=EXP)

            def emit_O(i):
                o, w = RC[i]
                for h in range(2):
                    vs = slice(h * 128, (h + 1) * 128)
                    ops = pp.tile([128, 512], f32, tag="o")
                    for mt in range(NMT):
                        nc.tensor.matmul(ops[:, :w], V[mt][:, vs],
                                         PT[mt][:, h, o:o + w],
                                         start=(mt == 0), stop=(mt == NMT - 1))
                    rcp = sb.tile([64, 512], f32, tag="rcp", bufs=4)
                    nc.vector.reciprocal_approx_fast(rcp[:, :w], ops[0:64, :w])
                    nc.vector.tensor_mul(OnT[h * 64:(h + 1) * 64, o:o + w],
                                         ops[64:128, :w], rcp[:, :w])
                # out projection for the r-tiles fully covered so far
                done = o + w
                while ystate[0] < NRT and min(ystate[0] * 128 + 128,
                                              R_PAD) <= done:
                    rt = ystate[0]
                    tw = min(128, R_PAD - rt * 128)
                    ps = pp.tile([128, 512], f32, tag="big")
                    rsl = slice(rt * 128, rt * 128 + tw)
                    nc.tensor.matmul(ps[:tw, :], OnT[:, rsl], wo,
                                     start=True, stop=True)
                    ysb = sb.tile([128, 512], bf16, tag="y", bufs=5)
                    nc.vector.tensor_copy(ysb[:tw, :], ps[:tw, :])
                    (nc.sync if rt % 2 == 0 else nc.gpsimd).dma_start(
                        out=y_d.ap()[rsl, :], in_=ysb[:tw, :])
                    ystate[0] += 1

            emit_S(0)
            for i in range(1, len(RC)):
                emit_Q(i)
                emit_S(i)
                emit_O(i - 1)
            emit_O(len(RC) - 1)

    nc.compile()
    return nc


def _prep(x, mask_np, mask_bert, W_qkv, W_out):
    """Host-side gather/shard. Returns (in_maps, meta)."""
    import ml_dtypes

    bf16 = ml_dtypes.bfloat16
    B, N, DIM = x.shape
    assert (B, DIM) == (2, _DIM)
    x = np.ascontiguousarray(x, dtype=np.float32)
    W_qkv = np.ascontiguousarray(W_qkv, dtype=np.float32)
    W_out = np.ascontiguousarray(W_out, dtype=np.float32)

    kv_idx, tail_idx, Ms, tails = [], [], [], []
    for b in range(B):
        npb = mask_np[b].astype(bool)
        bb = mask_bert[b].astype(bool)
        kv = np.nonzero(npb & ~bb)[0]
        tl = np.nonzero(npb & bb)[0]
        kv_idx.append(kv)
        tail_idx.append(tl)
        Ms.append(len(kv))
        tails.append(len(tl))

    M_PAD = max(128, _ceil_to(max(Ms), 128))
    # rows are packed [kv | tail] with no gap: the tail rows that fall in
    # [M_b, M_PAD) act as key/value candidates but are nulled by the kv
    # indicator (V rows scaled to 0, denominator contribution 0), so no
    # zero gap is needed and R_PAD shrinks to the real row count.
    R_PAD = max(128, _ceil_to(max(Ms[b] + tails[b] for b in range(B)), 16),
                M_PAD)

    NMT = M_PAD // 128
    xT_b, kvc_b, kvr_b, row_pos = [], [], [], []
    for b in range(B):
        xa = np.zeros((512, R_PAD), dtype=bf16)
        xa[:, :Ms[b]] = x[b][kv_idx[b]].T
        xa[:, Ms[b]:Ms[b] + tails[b]] = x[b][tail_idx[b]].T
        # pack into the on-chip tile layout [p, c, r] = xa[c*128+p, r] so
        # the DMA moves 4KB+ contiguous runs per partition
        xT_b.append(np.ascontiguousarray(
            xa.reshape(4, 128, R_PAD).transpose(1, 0, 2)))
        kvones = np.zeros(M_PAD, dtype=np.float32)
        kvones[:Ms[b]] = 1.0
        km = np.ascontiguousarray(kvones.reshape(NMT, 128).T)  # [128, NMT]
        kvc_b.append(km)
        kvr_b.append(np.repeat(km[:, :, None], 64, axis=2)
                     .reshape(128, NMT * 64).astype(bf16))
        # output row p of the device result corresponds to token row_pos[p]
        pos = np.concatenate([kv_idx[b], tail_idx[b]])
        row_pos.append(pos)

    def shuffle(wmat):
        # [512, 128] -> [128, 4*128] with [p, c*128+d] = wmat[c*128+p, d]
        return wmat.reshape(4, 128, 128).transpose(1, 0, 2).reshape(128, 512)

    scale = np.float32(_DH ** -0.5)
    in_maps = []
    for c in range(_CORES):
        b, g = divmod(c, 4)
        qc = slice(128 * g, 128 * g + 128)
        kc = slice(_INNER + 128 * g, _INNER + 128 * g + 128)
        vc = slice(2 * _INNER + 128 * g, 2 * _INNER + 128 * g + 128)
        wb1 = np.concatenate([
            shuffle(W_qkv[:, qc] * scale),
            shuffle(W_qkv[:, kc]),
        ], axis=1).astype(bf16)
        wb2 = np.concatenate([
            shuffle(W_qkv[:, vc]),
            W_out[128 * g:128 * g + 128, :],
            kvr_b[b].astype(np.float32),
        ], axis=1).astype(bf16)
        in_maps.append({"xT": xT_b[b], "wb1": np.ascontiguousarray(wb1),
                        "wb2": np.ascontiguousarray(wb2),
                        "kvc": kvc_b[b]})

    meta = dict(M_PAD=M_PAD, R_PAD=R_PAD, Ms=Ms, tails=tails,
                kv_idx=kv_idx, tail_idx=tail_idx, row_pos=row_pos)
    return in_maps, meta


def _assemble(results, meta, x, mask_np, W_qkv, W_out, b_out):
    B, N, _ = x.shape
    out = np.empty((B, N, _DIM), dtype=np.float32)
    Wv_full = W_qkv[:, 2 * _INNER:].astype(np.float32)
    for b in range(B):
        # constant output for fully-masked rows: uniform attention = mean(V)
        meanv = (x[b].mean(axis=0, dtype=np.float32) @ Wv_full)
        yconst = meanv @ W_out.astype(np.float32) + b_out
        out[b, :, :] = yconst[None, :]
        Mb, tb = meta["Ms"][b], meta["tails"][b]
        if Mb == 0:
            # no unmasked kv columns: every row is fully masked -> uniform
            continue
        acc = None
        for g in range(4):
            yp = np.asarray(results[4 * b + g]["y"], dtype=np.float32)
            acc = yp if acc is None else acc + yp
        out[b, meta["row_pos"][b], :] = acc[:Mb + tb] + b_out
    return out


_CACHE = {}


def _get_bass(R_PAD, M_PAD):
    key = (R_PAD, M_PAD)
    if key not in _CACHE:
        _CACHE[key] = build_bass(R_PAD, M_PAD)
    return _CACHE[key]


def run_spmd(in_maps, meta, trace=False, tmpdir=None, trace_cores=None):
    from concourse.bass_utils import run_bass_kernel_spmd

    nc = _get_bass(meta["R_PAD"], meta["M_PAD"])
    return run_bass_kernel_spmd(
        nc, in_maps, core_ids=list(range(_CORES)), trace=trace, tmpdir=tmpdir,
        trace_cores=trace_cores)


def kernel(x, mask_np, mask_bert, W_qkv, W_out, b_out):
    x = np.asarray(x)
    mask_np = np.asarray(mask_np)
    mask_bert = np.asarray(mask_bert)
    W_qkv = np.asarray(W_qkv, dtype=np.float32)
    W_out = np.asarray(W_out, dtype=np.float32)
    b_out = np.asarray(b_out, dtype=np.float32)

    in_maps, meta = _prep(x, mask_np, mask_bert, W_qkv, W_out)
    res = run_spmd(in_maps, meta)
    return _assemble(res.results, meta, x, mask_np, W_qkv, W_out, b_out)
